# revision 16
# baseline (speedup 1.0000x reference)
"""MoE top-2 routing kernel for 8 Trainium2 NeuronCores (v3).

Problem (hardcoded shapes): x [64,8,2048] f32, gate_w [2048,8] f32,
w1/w3 [8,2048,4096] f32, w2 [8,4096,2048] f32, top_k=2.

Strategy (expert parallelism, half-int8-compressed gate/up weights):
  - Host computes the gate (512x8 logits, top-2, softmax) exactly as the
    reference does; tokens are dispatched per expert (one expert per
    NeuronCore), padded to capacity C=144 (deterministic max count 140).
  - Half of the w1/w3 output columns (the even 256-col groups) are
    quantized to int8 with one scale per output column f
    (s1[f] = max_d |w[d,f]| / 127), cutting w13 HBM traffic by 25%;
    the other half stays bf16.  The two int8 m-tiles of a group are
    packed into int16 words (lo byte biased +128, hi byte signed); on
    device the DVE unpacks them with 16-bit tensor_scalar ops (AND 0xFF
    / AND 0xFF00 then *1/256 cast, ~0.43us per 128x1024 op) and the
    Scalar engine converts the lo plane (Copy with bias=-128), so the
    dequant runs ~4x faster than a direct int8 CAST and stays off the
    PE critical path.
  - s1 is applied for free inside the silu activation (func(in*scale),
    scale = 1.0 for the bf16 groups), s3 is folded into w2 on the host
    (w2' = diag(s3) @ w2, bf16), so dequantization adds no other ops.
  - Stage-1/2 PSUM groups use 4 banks each so two groups ping-pong
    across the 8 banks: the activation/copy drain of group g overlaps
    the accumulation of group g+1 and the PE never stalls.
  - outT is returned in bf16; the combine weights are folded into the
    host-side scatter-add.
Measured rel err vs the fp32 reference: ~9.2e-3 (gate 2e-2).
"""

import numpy as np

B, S, D, F, E = 64, 8, 2048, 4096, 8
T = B * S  # 512 tokens
P = 128
KD = D // P    # 16 k-tiles, D contraction (stage 1)
KF = F // P    # 32 k-tiles, F contraction (stage 2)
MF = F // P    # 32 m-tiles, stage 1
MD = D // P    # 16 m-tiles, stage 2
G1 = 2         # stage-1 m-tiles per group (2 gate + 2 up = 4 PSUM banks)
G2 = 4         # stage-2 m-tiles per group (4 PSUM banks)
KQ1 = 4        # stage-1 k-tiles per weight block (one DMA each)
KQ2 = 4        # stage-2 k-tiles per weight block
NG1 = MF // G1          # 16 stage-1 groups (even: int8, odd: bf16)
NG2 = MD // G2          # 4 stage-2 groups
NB1 = KD // KQ1         # 4 w13 blocks per stage-1 group
NB2 = KF // KQ2         # 8 w2 blocks per stage-2 group

_cache = {}
last_results = None  # BassKernelResults of the most recent device run


def _bf16():
    import ml_dtypes
    return np.dtype(ml_dtypes.bfloat16)


def _build(C):
    import concourse.mybir as mybir
    import concourse.tile as tile
    from concourse import bacc

    nc = bacc.Bacc(None, target_bir_lowering=False)
    f32 = mybir.dt.float32
    bf16 = mybir.dt.bfloat16
    i16 = mybir.dt.int16

    NEG = NG1 // 2  # 8 even (int8) groups, 8 odd (bf16) groups
    # w13q block (ge, b): [128 part, kk, w=2, 128] int16 words packing the
    # two m-tiles of the group (lo byte = m0 biased +128, hi = m1 signed)
    w13q = nc.declare_dram_parameter("w13q", [NEG, NB1, P, KQ1, 2, P],
                                     i16, isOutput=False)
    # w13b block (go, b): same layout in bf16 (4KB lines)
    w13b = nc.declare_dram_parameter("w13b", [NEG, NB1, P, KQ1, 2, G1 * P],
                                     bf16, isOutput=False)
    # w2p block (g2, b): [128 part, kk, G2*128] bf16 (4KB lines), s3 folded
    w2p = nc.declare_dram_parameter("w2p", [NG2, NB2, P, KQ2, G2 * P],
                                    bf16, isOutput=False)
    xT = nc.declare_dram_parameter("xT", [P, KD, C], bf16, isOutput=False)
    s1p = nc.declare_dram_parameter("s1p", [P, MF], f32, isOutput=False)
    outT = nc.declare_dram_parameter("outT", [NG2, P, G2, C], bf16,
                                     isOutput=True)

    with tile.TileContext(nc) as tc:
        with (
            tc.tile_pool(name="xpool", bufs=1) as xpool,
            tc.tile_pool(name="hpool", bufs=1) as hpool,
            tc.tile_pool(name="w8pool", bufs=10) as w8pool,
            tc.tile_pool(name="tmpool", bufs=12) as tmpool,
            tc.tile_pool(name="w16pool", bufs=10) as w16pool,
            tc.tile_pool(name="w2pool", bufs=8) as w2pool,
            tc.tile_pool(name="psum", bufs=8, space="PSUM") as psum,
            tc.tile_pool(name="spool", bufs=4) as spool,
            tc.tile_pool(name="opool", bufs=2) as opool,
        ):
            xt = xpool.tile([P, KD, C], bf16)
            nc.sync.dma_start(out=xt[:, 0:4, :], in_=xT[:, 0:4, :])
            nc.scalar.dma_start(out=xt[:, 4:, :], in_=xT[:, 4:, :])
            s1t = xpool.tile([P, MF], f32, name="s1t")
            nc.sync.dma_start(out=s1t[:], in_=s1p[:])
            ht = hpool.tile([P, KF, C], bf16)

            dma_eng = [nc.sync, nc.scalar]
            ndma = 0
            Alu = mybir.AluOpType

            def unpack(dst, srcw):
                # dst [P, 2, KQ1, 2, P] bf16: plane 0 = m0 (lo bytes),
                # plane 1 = m1 (hi bytes).  srcw [P, KQ1, 2, P] int16.
                # All 4 ops on the DVE so the Scalar engine keeps only the
                # silu drain (PSUM banks free without queueing delays).
                tA = tmpool.tile([P, KQ1, 2, P], i16, tag="tA")
                tB = tmpool.tile([P, KQ1, 2, P], i16, tag="tB")
                nc.vector.tensor_scalar(tA[:], srcw[:], 255, None,
                                        Alu.bitwise_and)
                nc.vector.tensor_scalar(tB[:], srcw[:], -256, None,
                                        Alu.bitwise_and)
                nc.vector.tensor_scalar(dst[:, 0], tA[:], 1.0, -128.0,
                                        Alu.mult, Alu.add)
                nc.vector.tensor_scalar(dst[:, 1], tB[:], 1.0 / 256, None,
                                        Alu.mult)

            warm = xpool.tile([P, 256], bf16, name="warm")
            nc.vector.memset(warm[:], 0.0)
            ps_w = psum.tile([P, C], f32, tag="ps", name="ps_warm")
            for i in range(40):
                nc.tensor.matmul(ps_w[:], warm[:, :P], warm[:, :C],
                                 start=True, stop=True)

            # stage 1: hT[f, t] = silu(s1 * (w1q^T xT)) * (w3q^T xT)
            for g in range(NG1):
                ps_g = [psum.tile([P, C], f32, tag="ps", name=f"ps_g{g}_{m}")
                        for m in range(G1)]
                ps_u = [psum.tile([P, C], f32, tag="ps", name=f"ps_u{g}_{m}")
                        for m in range(G1)]
                for b in range(NB1):
                    if g % 2 == 0:
                        w8 = w8pool.tile([P, KQ1, 2, P], i16, tag="w8")
                        dma_eng[ndma % 2].dma_start(out=w8[:],
                                                    in_=w13q[g // 2, b])
                        ndma += 1
                        w16 = w16pool.tile([P, 2, KQ1, 2, P], bf16,
                                           tag="w16")
                        unpack(w16, w8)

                        def lhs1(kk, w_, m, t=w16):
                            return t[:, m, kk, w_, :]
                    else:
                        w16 = w16pool.tile([P, KQ1, 2, G1 * P], bf16,
                                           tag="w16")
                        dma_eng[ndma % 2].dma_start(out=w16[:],
                                                    in_=w13b[g // 2, b])
                        ndma += 1

                        def lhs1(kk, w_, m, t=w16):
                            return t[:, kk, w_, m * P:(m + 1) * P]
                    for kk in range(KQ1):
                        k = b * KQ1 + kk
                        st, sp = (k == 0), (k == KD - 1)
                        for m in range(G1):
                            nc.tensor.matmul(ps_g[m][:], lhs1(kk, 0, m),
                                             xt[:, k, :], start=st, stop=sp)
                            nc.tensor.matmul(ps_u[m][:], lhs1(kk, 1, m),
                                             xt[:, k, :], start=st, stop=sp)
                # drain ops free this group's PSUM banks; schedule them
                # ahead of prefetched unpack work in the engine FIFOs
                with tc.high_priority(offset=200):
                    for m in range(G1):
                        mf = g * G1 + m
                        sig = spool.tile([P, C], f32, tag="sig")
                        nc.scalar.activation(sig[:], ps_g[m][:],
                                             mybir.ActivationFunctionType.Silu,
                                             scale=s1t[:, mf:mf + 1])
                        nc.vector.tensor_tensor(out=ht[:, mf, :], in0=sig[:],
                                                in1=ps_u[m][:],
                                                op=mybir.AluOpType.mult)

            # stage 2: outT[d, t] = w2'^T @ hT
            for g in range(NG2):
                ps_o = [psum.tile([P, C], f32, tag="ps", name=f"ps_o{g}_{m}")
                        for m in range(G2)]
                for b in range(NB2):
                    w2t = w2pool.tile([P, KQ2, G2 * P], bf16, tag="w2")
                    dma_eng[ndma % 2].dma_start(out=w2t[:], in_=w2p[g, b])
                    ndma += 1
                    for kk in range(KQ2):
                        k = b * KQ2 + kk
                        st, sp = (k == 0), (k == KF - 1)
                        for m in range(G2):
                            nc.tensor.matmul(ps_o[m][:],
                                             w2t[:, kk, m * P:(m + 1) * P],
                                             ht[:, k, :], start=st, stop=sp)
                obuf = opool.tile([P, G2, C], bf16, tag="o", name=f"ob{g}")
                with tc.high_priority(offset=200):
                    for m in range(G2):
                        nc.vector.tensor_copy(out=obuf[:, m, :],
                                              in_=ps_o[m][:])
                nc.sync.dma_start(out=outT[g], in_=obuf[:])

    nc.compile()
    return nc


def _route(x2d, gate_w, top_k):
    """Replicates the reference gate on host: returns (sel [T,k], cw [T,k])."""
    logits = x2d @ gate_w                       # [T, E] fp32
    sel = np.argsort(-logits, axis=-1, kind="stable")[:, :top_k]
    vals = np.take_along_axis(logits, sel, axis=-1)
    m = vals.max(axis=-1, keepdims=True)
    ex = np.exp(vals - m)
    cw = ex / ex.sum(axis=-1, keepdims=True)
    return sel, cw


def _quant_pack(w1, w3, w2):
    """Per-expert: int8-quantize the even 256-col groups of w1/w3 (one scale
    per output column), keep odd groups bf16, fold s3 into w2.

    Returns (w13q, w13b, w2p, s1p) in the device DMA layouts."""
    bf16 = _bf16()
    GC = G1 * P  # 256 cols per group
    fcols = np.arange(F)
    even = ((fcols // GC) % 2) == 0
    out = []
    for e in range(E):
        w1e, w3e = w1[e], w3[e]                      # [D, F]
        s1 = np.abs(w1e).max(axis=0) / 127.0         # [F]
        s3 = np.abs(w3e).max(axis=0) / 127.0
        q1 = np.clip(np.rint(w1e / s1), -127, 127).astype(np.int8)
        q3 = np.clip(np.rint(w3e / s3), -127, 127).astype(np.int8)
        # int8 halves on even columns, packed as int16 words pairing the
        # two m-tiles of each group: lo byte = m0 (biased +128), hi = m1
        # (signed).  [w, D, F/2] -> [w, b, kk, p, ge, m, c]
        qs = np.stack([q1[:, even], q3[:, even]], 0)
        qs = qs.reshape(2, NB1, KQ1, P, NG1 // 2, G1, P)
        lo = (qs[..., 0, :].astype(np.int16) + 128).astype(np.uint16) & 0xFF
        hi = qs[..., 1, :].astype(np.int16).astype(np.uint16) << 8
        words = (hi | lo).view(np.int16)           # [w, b, kk, p, ge, c]
        w13q = np.ascontiguousarray(words.transpose(4, 1, 3, 2, 0, 5))
        # bf16 halves on odd columns: h there is true-scale (s1p = s3_eff = 1)
        bs = np.stack([w1e[:, ~even].astype(bf16), w3e[:, ~even].astype(bf16)],
                      0)
        bs = bs.reshape(2, NB1, KQ1, P, NG1 // 2, GC)
        w13b = np.ascontiguousarray(bs.transpose(4, 1, 3, 2, 0, 5))
        # s1 scale vector: even groups s1, odd groups 1.0
        s1v = np.where(even, s1, 1.0).astype(np.float32)
        s1p = np.ascontiguousarray(s1v.reshape(MF, P).T)  # [P, MF]
        # w2' = diag(s3_eff) @ w2, bf16; s3_eff = s3 on even cols, 1 on odd
        s3v = np.where(even, s3, 1.0).astype(np.float32)
        w2f = (w2[e] * s3v[:, None]).astype(bf16)
        w2r = w2f.reshape(NB2, KQ2, P, NG2, G2 * P)
        w2p = np.ascontiguousarray(w2r.transpose(3, 0, 2, 1, 4))
        out.append((w13q, w13b, w2p, s1p))
    return out


def kernel(x, gate_w, w1, w3, w2, top_k):
    from concourse.bass_utils import run_bass_kernel_spmd

    x = np.asarray(x, np.float32)
    gate_w = np.asarray(gate_w, np.float32)
    w1 = np.asarray(w1, np.float32)
    w3 = np.asarray(w3, np.float32)
    w2 = np.asarray(w2, np.float32)
    k = int(top_k)
    bf16 = _bf16()

    x2d = x.reshape(T, D)
    sel, cw = _route(x2d, gate_w, k)

    # token lists per expert
    idx = [np.where((sel == e).any(axis=1))[0] for e in range(E)]
    wgt = []
    for e in range(E):
        m = sel[idx[e]] == e
        wgt.append(cw[idx[e]][m].astype(np.float32))
    counts = np.array([len(i) for i in idx])
    maxc = int(counts.max())
    C = max(140, -(-maxc // 4) * 4)
    n_chunks = 1
    if C > 512:  # capacity overflow: run multiple passes of 512
        C = 512
        n_chunks = -(-maxc // C)

    if C not in _cache:
        _cache[C] = _build(C)
    nc = _cache[C]

    wpacked = _quant_pack(w1, w3, w2)

    out = np.zeros((T, D), np.float32)
    for chunk in range(n_chunks):
        in_maps = []
        for e in range(E):
            ide = idx[e][chunk * C:(chunk + 1) * C]
            xTe = np.zeros((D, C), bf16)
            xTe[:, :len(ide)] = x2d[ide].T.astype(bf16)
            in_maps.append({
                "xT": np.ascontiguousarray(
                    xTe.reshape(KD, P, C).transpose(1, 0, 2)),
                "w13q": wpacked[e][0],
                "w13b": wpacked[e][1],
                "w2p": wpacked[e][2],
                "s1p": wpacked[e][3],
            })
        res = run_bass_kernel_spmd(nc, in_maps, core_ids=list(range(E)))
        global last_results
        last_results = res
        for e in range(E):
            ide = idx[e][chunk * C:(chunk + 1) * C]
            if len(ide) == 0:
                continue
            we = wgt[e][chunk * C:(chunk + 1) * C]
            # outT [NG2, P, G2, C] -> [D, C] with d = g*G2*P + m*P + p
            oTe = res.results[e]["outT"].astype(np.float32)
            oTe = oTe.transpose(0, 2, 1, 3).reshape(D, C)
            # token indices are unique within one expert's list
            out[ide] += we[:, None] * oTe[:, :len(ide)].T

    return out.reshape(B, S, D)


# revision 18
# speedup vs baseline: 1.0499x; 1.0499x over previous
"""MoE top-2 routing kernel for 8 Trainium2 NeuronCores (v3).

Problem (hardcoded shapes): x [64,8,2048] f32, gate_w [2048,8] f32,
w1/w3 [8,2048,4096] f32, w2 [8,4096,2048] f32, top_k=2.

Strategy (expert parallelism, half-int8-compressed gate/up weights):
  - Host computes the gate (512x8 logits, top-2, softmax) exactly as the
    reference does; tokens are dispatched per expert (one expert per
    NeuronCore), padded to capacity C=144 (deterministic max count 140).
  - Half of the w1/w3 output columns (the even 256-col groups) are
    quantized to int8 with one scale per output column f
    (s1[f] = max_d |w[d,f]| / 127), cutting w13 HBM traffic by 25%;
    the other half stays bf16.  The two int8 m-tiles of a group are
    packed into int16 words (lo byte biased +128, hi byte signed); the
    DVE unpacks them with 16-bit tensor_scalar ops (AND 0xFF / AND
    0xFF00, then mult/add casts to bf16, ~0.43us per 128x1024 op), ~4x
    faster than a direct int8 CAST, keeping the dequant off the PE
    critical path.  The Scalar engine keeps only the silu drain and the
    PSUM-freeing ops run at high priority, so group transitions do not
    stall the PE behind prefetched unpack work.
  - s1 is applied for free inside the silu activation (func(in*scale),
    scale = 1.0 for the bf16 groups), s3 is folded into w2 on the host
    (w2' = diag(s3) @ w2, bf16), so dequantization adds no other ops.
  - Stage-1/2 PSUM groups use 4 banks each so two groups ping-pong
    across the 8 banks: the activation/copy drain of group g overlaps
    the accumulation of group g+1 and the PE never stalls.
  - outT is returned in bf16; the combine weights are folded into the
    host-side scatter-add.
Measured rel err vs the fp32 reference: ~9.2e-3 (gate 2e-2).
"""

import numpy as np

B, S, D, F, E = 64, 8, 2048, 4096, 8
T = B * S  # 512 tokens
P = 128
KD = D // P    # 16 k-tiles, D contraction (stage 1)
KF = F // P    # 32 k-tiles, F contraction (stage 2)
MF = F // P    # 32 m-tiles, stage 1
MD = D // P    # 16 m-tiles, stage 2
G1 = 2         # stage-1 m-tiles per group (2 gate + 2 up = 4 PSUM banks)
G2 = 4         # stage-2 m-tiles per group (4 PSUM banks)
KQ1 = 4        # stage-1 k-tiles per weight block (one DMA each)
KQ2 = 4        # stage-2 k-tiles per weight block
NG1 = MF // G1          # 16 stage-1 groups (odd: int8, even: bf16)
NG2 = MD // G2          # 4 stage-2 groups
NB1 = KD // KQ1         # 4 w13 blocks per stage-1 group
NB2 = KF // KQ2         # 8 w2 blocks per stage-2 group

_cache = {}
last_results = None  # BassKernelResults of the most recent device run


def _bf16():
    import ml_dtypes
    return np.dtype(ml_dtypes.bfloat16)


def _build(C):
    import concourse.mybir as mybir
    import concourse.tile as tile
    from concourse import bacc

    nc = bacc.Bacc(None, target_bir_lowering=False)
    f32 = mybir.dt.float32
    bf16 = mybir.dt.bfloat16
    i16 = mybir.dt.int16

    NEG = NG1 // 2  # 8 even (int8) groups, 8 odd (bf16) groups
    # w13q block (ge, b): [128 part, kk, w=2, 128] int16 words packing the
    # two m-tiles of the group (lo byte = m0 biased +128, hi = m1 signed)
    w13q = nc.declare_dram_parameter("w13q", [NEG, NB1, P, KQ1, 2, P],
                                     i16, isOutput=False)
    # w13b block (go, b): same layout in bf16 (4KB lines)
    w13b = nc.declare_dram_parameter("w13b", [NEG, NB1, P, KQ1, 2, G1 * P],
                                     bf16, isOutput=False)
    # w2p block (g2, b): [128 part, kk, G2*128] bf16 (4KB lines), s3 folded
    w2p = nc.declare_dram_parameter("w2p", [NG2, NB2, P, KQ2, G2 * P],
                                    bf16, isOutput=False)
    xT = nc.declare_dram_parameter("xT", [P, KD, C], bf16, isOutput=False)
    s1p = nc.declare_dram_parameter("s1p", [P, MF], f32, isOutput=False)
    outT = nc.declare_dram_parameter("outT", [NG2, P, G2, C], bf16,
                                     isOutput=True)

    with tile.TileContext(nc) as tc:
        with (
            tc.tile_pool(name="xpool", bufs=1) as xpool,
            tc.tile_pool(name="hpool", bufs=1) as hpool,
            tc.tile_pool(name="w8pool", bufs=10) as w8pool,
            tc.tile_pool(name="tmpool", bufs=12) as tmpool,
            tc.tile_pool(name="w16pool", bufs=12) as w16pool,
            tc.tile_pool(name="w2pool", bufs=14) as w2pool,
            tc.tile_pool(name="psum", bufs=8, space="PSUM") as psum,
            tc.tile_pool(name="spool", bufs=4) as spool,
            tc.tile_pool(name="opool", bufs=2) as opool,
        ):
            xt = xpool.tile([P, KD, C], bf16)
            nc.sync.dma_start(out=xt[:, 0:4, :], in_=xT[:, 0:4, :])
            nc.scalar.dma_start(out=xt[:, 4:, :], in_=xT[:, 4:, :])
            s1t = xpool.tile([P, MF], f32, name="s1t")
            nc.sync.dma_start(out=s1t[:], in_=s1p[:])
            ht = hpool.tile([P, KF, C], bf16)

            dma_eng = [nc.sync, nc.scalar]
            ndma = 0
            Alu = mybir.AluOpType

            def unpack(dst, srcw):
                # dst [P, 2, KQ1, 2, P] bf16: plane 0 = m0 (lo bytes),
                # plane 1 = m1 (hi bytes).  srcw [P, KQ1, 2, P] int16.
                # All 4 ops on the DVE so the Scalar engine keeps only the
                # silu drain (PSUM banks free without queueing delays).
                tA = tmpool.tile([P, KQ1, 2, P], i16, tag="tA")
                tB = tmpool.tile([P, KQ1, 2, P], i16, tag="tB")
                nc.vector.tensor_scalar(tA[:], srcw[:], 255, None,
                                        Alu.bitwise_and)
                nc.vector.tensor_scalar(tB[:], srcw[:], -256, None,
                                        Alu.bitwise_and)
                nc.vector.tensor_scalar(dst[:, 0], tA[:], 1.0, -128.0,
                                        Alu.mult, Alu.add)
                nc.vector.tensor_scalar(dst[:, 1], tB[:], 1.0 / 256, None,
                                        Alu.mult)

            warm = xpool.tile([P, 256], bf16, name="warm")
            nc.vector.memset(warm[:], 0.0)
            ps_w = psum.tile([P, C], f32, tag="ps", name="ps_warm")
            for i in range(64):
                nc.tensor.matmul(ps_w[:], warm[:, :P], warm[:, :C],
                                 start=True, stop=True)

            # stage 1: hT[f, t] = silu(s1 * (w1q^T xT)) * (w3q^T xT)
            for g in range(NG1):
                ps_g = [psum.tile([P, C], f32, tag="ps", name=f"ps_g{g}_{m}")
                        for m in range(G1)]
                ps_u = [psum.tile([P, C], f32, tag="ps", name=f"ps_u{g}_{m}")
                        for m in range(G1)]
                for b in range(NB1):
                    if g % 2 == 1:
                        w8 = w8pool.tile([P, KQ1, 2, P], i16, tag="w8")
                        dma_eng[ndma % 2].dma_start(out=w8[:],
                                                    in_=w13q[g // 2, b])
                        ndma += 1
                        w16 = w16pool.tile([P, 2, KQ1, 2, P], bf16,
                                           tag="w16")
                        unpack(w16, w8)

                        def lhs1(kk, w_, m, t=w16):
                            return t[:, m, kk, w_, :]
                    else:
                        w16 = w16pool.tile([P, KQ1, 2, G1 * P], bf16,
                                           tag="w16")
                        dma_eng[ndma % 2].dma_start(out=w16[:],
                                                    in_=w13b[g // 2, b])
                        ndma += 1

                        def lhs1(kk, w_, m, t=w16):
                            return t[:, kk, w_, m * P:(m + 1) * P]
                    for kk in range(KQ1):
                        k = b * KQ1 + kk
                        st, sp = (k == 0), (k == KD - 1)
                        for m in range(G1):
                            nc.tensor.matmul(ps_g[m][:], lhs1(kk, 0, m),
                                             xt[:, k, :], start=st, stop=sp)
                            nc.tensor.matmul(ps_u[m][:], lhs1(kk, 1, m),
                                             xt[:, k, :], start=st, stop=sp)
                # drain ops free this group's PSUM banks; schedule them
                # ahead of prefetched unpack work in the engine FIFOs
                with tc.high_priority(offset=200):
                    for m in range(G1):
                        mf = g * G1 + m
                        sig = spool.tile([P, C], f32, tag="sig")
                        nc.scalar.activation(sig[:], ps_g[m][:],
                                             mybir.ActivationFunctionType.Silu,
                                             scale=s1t[:, mf:mf + 1])
                        nc.vector.tensor_tensor(out=ht[:, mf, :], in0=sig[:],
                                                in1=ps_u[m][:],
                                                op=mybir.AluOpType.mult)

            # stage 2: outT[d, t] = w2'^T @ hT
            for g in range(NG2):
                ps_o = [psum.tile([P, C], f32, tag="ps", name=f"ps_o{g}_{m}")
                        for m in range(G2)]
                for b in range(NB2):
                    w2t = w2pool.tile([P, KQ2, G2 * P], bf16, tag="w2")
                    dma_eng[ndma % 2].dma_start(out=w2t[:], in_=w2p[g, b])
                    ndma += 1
                    for kk in range(KQ2):
                        k = b * KQ2 + kk
                        st, sp = (k == 0), (k == KF - 1)
                        for m in range(G2):
                            nc.tensor.matmul(ps_o[m][:],
                                             w2t[:, kk, m * P:(m + 1) * P],
                                             ht[:, k, :], start=st, stop=sp)
                obuf = opool.tile([P, G2, C], bf16, tag="o", name=f"ob{g}")
                with tc.high_priority(offset=200):
                    for m in range(G2):
                        nc.vector.tensor_copy(out=obuf[:, m, :],
                                              in_=ps_o[m][:])
                nc.sync.dma_start(out=outT[g], in_=obuf[:])

    nc.compile()
    return nc


def _route(x2d, gate_w, top_k):
    """Replicates the reference gate on host: returns (sel [T,k], cw [T,k])."""
    logits = x2d @ gate_w                       # [T, E] fp32
    sel = np.argsort(-logits, axis=-1, kind="stable")[:, :top_k]
    vals = np.take_along_axis(logits, sel, axis=-1)
    m = vals.max(axis=-1, keepdims=True)
    ex = np.exp(vals - m)
    cw = ex / ex.sum(axis=-1, keepdims=True)
    return sel, cw


def _quant_pack(w1, w3, w2):
    """Per-expert: int8-quantize the even 256-col groups of w1/w3 (one scale
    per output column), keep odd groups bf16, fold s3 into w2.

    Returns (w13q, w13b, w2p, s1p) in the device DMA layouts."""
    bf16 = _bf16()
    GC = G1 * P  # 256 cols per group
    fcols = np.arange(F)
    even = ((fcols // GC) % 2) == 1
    out = []
    for e in range(E):
        w1e, w3e = w1[e], w3[e]                      # [D, F]
        s1 = np.abs(w1e).max(axis=0) / 127.0         # [F]
        s3 = np.abs(w3e).max(axis=0) / 127.0
        q1 = np.clip(np.rint(w1e / s1), -127, 127).astype(np.int8)
        q3 = np.clip(np.rint(w3e / s3), -127, 127).astype(np.int8)
        # int8 halves on even columns, packed as int16 words pairing the
        # two m-tiles of each group: lo byte = m0 (biased +128), hi = m1
        # (signed).  [w, D, F/2] -> [w, b, kk, p, ge, m, c]
        qs = np.stack([q1[:, even], q3[:, even]], 0)
        qs = qs.reshape(2, NB1, KQ1, P, NG1 // 2, G1, P)
        lo = (qs[..., 0, :].astype(np.int16) + 128).astype(np.uint16) & 0xFF
        hi = qs[..., 1, :].astype(np.int16).astype(np.uint16) << 8
        words = (hi | lo).view(np.int16)           # [w, b, kk, p, ge, c]
        w13q = np.ascontiguousarray(words.transpose(4, 1, 3, 2, 0, 5))
        # bf16 halves on odd columns: h there is true-scale (s1p = s3_eff = 1)
        bs = np.stack([w1e[:, ~even].astype(bf16), w3e[:, ~even].astype(bf16)],
                      0)
        bs = bs.reshape(2, NB1, KQ1, P, NG1 // 2, GC)
        w13b = np.ascontiguousarray(bs.transpose(4, 1, 3, 2, 0, 5))
        # s1 scale vector: even groups s1, odd groups 1.0
        s1v = np.where(even, s1, 1.0).astype(np.float32)
        s1p = np.ascontiguousarray(s1v.reshape(MF, P).T)  # [P, MF]
        # w2' = diag(s3_eff) @ w2, bf16; s3_eff = s3 on even cols, 1 on odd
        s3v = np.where(even, s3, 1.0).astype(np.float32)
        w2f = (w2[e] * s3v[:, None]).astype(bf16)
        w2r = w2f.reshape(NB2, KQ2, P, NG2, G2 * P)
        w2p = np.ascontiguousarray(w2r.transpose(3, 0, 2, 1, 4))
        out.append((w13q, w13b, w2p, s1p))
    return out


def kernel(x, gate_w, w1, w3, w2, top_k):
    from concourse.bass_utils import run_bass_kernel_spmd

    x = np.asarray(x, np.float32)
    gate_w = np.asarray(gate_w, np.float32)
    w1 = np.asarray(w1, np.float32)
    w3 = np.asarray(w3, np.float32)
    w2 = np.asarray(w2, np.float32)
    k = int(top_k)
    bf16 = _bf16()

    x2d = x.reshape(T, D)
    sel, cw = _route(x2d, gate_w, k)

    # token lists per expert
    idx = [np.where((sel == e).any(axis=1))[0] for e in range(E)]
    wgt = []
    for e in range(E):
        m = sel[idx[e]] == e
        wgt.append(cw[idx[e]][m].astype(np.float32))
    counts = np.array([len(i) for i in idx])
    maxc = int(counts.max())
    C = max(140, -(-maxc // 4) * 4)
    n_chunks = 1
    if C > 512:  # capacity overflow: run multiple passes of 512
        C = 512
        n_chunks = -(-maxc // C)

    if C not in _cache:
        _cache[C] = _build(C)
    nc = _cache[C]

    wpacked = _quant_pack(w1, w3, w2)

    out = np.zeros((T, D), np.float32)
    for chunk in range(n_chunks):
        in_maps = []
        for e in range(E):
            ide = idx[e][chunk * C:(chunk + 1) * C]
            xTe = np.zeros((D, C), bf16)
            xTe[:, :len(ide)] = x2d[ide].T.astype(bf16)
            in_maps.append({
                "xT": np.ascontiguousarray(
                    xTe.reshape(KD, P, C).transpose(1, 0, 2)),
                "w13q": wpacked[e][0],
                "w13b": wpacked[e][1],
                "w2p": wpacked[e][2],
                "s1p": wpacked[e][3],
            })
        res = run_bass_kernel_spmd(nc, in_maps, core_ids=list(range(E)))
        global last_results
        last_results = res
        for e in range(E):
            ide = idx[e][chunk * C:(chunk + 1) * C]
            if len(ide) == 0:
                continue
            we = wgt[e][chunk * C:(chunk + 1) * C]
            # outT [NG2, P, G2, C] -> [D, C] with d = g*G2*P + m*P + p
            oTe = res.results[e]["outT"].astype(np.float32)
            oTe = oTe.transpose(0, 2, 1, 3).reshape(D, C)
            # token indices are unique within one expert's list
            out[ide] += we[:, None] * oTe[:, :len(ide)].T

    return out.reshape(B, S, D)


# revision 19
# speedup vs baseline: 1.0865x; 1.0349x over previous
"""MoE top-2 routing kernel for 8 Trainium2 NeuronCores (v3).

Problem (hardcoded shapes): x [64,8,2048] f32, gate_w [2048,8] f32,
w1/w3 [8,2048,4096] f32, w2 [8,4096,2048] f32, top_k=2.

Strategy (expert parallelism, half-int8-compressed gate/up weights):
  - Host computes the gate (512x8 logits, top-2, softmax) exactly as the
    reference does; tokens are dispatched per expert (one expert per
    NeuronCore), padded to capacity C=144 (deterministic max count 140).
  - Half of the w1/w3 output columns (the even 256-col groups) are
    quantized to int8 with one scale per output column f
    (s1[f] = max_d |w[d,f]| / 127), cutting w13 HBM traffic by 25%;
    the other half stays bf16.  The two int8 m-tiles of a group are
    packed into int16 words (lo byte biased +128, hi byte signed); the
    DVE unpacks them with 16-bit tensor_scalar ops (AND 0xFF / AND
    0xFF00, then mult/add casts to bf16, ~0.43us per 128x1024 op), ~4x
    faster than a direct int8 CAST, keeping the dequant off the PE
    critical path.  The Scalar engine keeps only the silu drain and the
    PSUM-freeing ops run at high priority, so group transitions do not
    stall the PE behind prefetched unpack work.
  - s1 is applied for free inside the silu activation (func(in*scale),
    scale = 1.0 for the bf16 groups), s3 is folded into w2 on the host
    (w2' = diag(s3) @ w2, bf16), so dequantization adds no other ops.
  - Stage-1/2 PSUM groups use 4 banks each so two groups ping-pong
    across the 8 banks: the activation/copy drain of group g overlaps
    the accumulation of group g+1 and the PE never stalls.
  - outT is returned in bf16; the combine weights are folded into the
    host-side scatter-add.
Measured rel err vs the fp32 reference: ~9.2e-3 (gate 2e-2).
"""

import numpy as np

B, S, D, F, E = 64, 8, 2048, 4096, 8
T = B * S  # 512 tokens
P = 128
KD = D // P    # 16 k-tiles, D contraction (stage 1)
KF = F // P    # 32 k-tiles, F contraction (stage 2)
MF = F // P    # 32 m-tiles, stage 1
MD = D // P    # 16 m-tiles, stage 2
G1 = 2         # stage-1 m-tiles per group (2 gate + 2 up = 4 PSUM banks)
G2 = 4         # stage-2 m-tiles per group (4 PSUM banks)
KQ1 = 4        # stage-1 k-tiles per weight block (one DMA each)
KQ2 = 4        # stage-2 k-tiles per weight block
NG1 = MF // G1          # 16 stage-1 groups (odd: int8, even: bf16)
NG2 = MD // G2          # 4 stage-2 groups
NB1 = KD // KQ1         # 4 w13 blocks per stage-1 group
NB2 = KF // KQ2         # 8 w2 blocks per stage-2 group

_cache = {}
last_results = None  # BassKernelResults of the most recent device run


def _bf16():
    import ml_dtypes
    return np.dtype(ml_dtypes.bfloat16)


def _build(C):
    import concourse.mybir as mybir
    import concourse.tile as tile
    from concourse import bacc

    nc = bacc.Bacc(None, target_bir_lowering=False)
    f32 = mybir.dt.float32
    bf16 = mybir.dt.bfloat16
    i16 = mybir.dt.int16

    NEG = NG1 // 2  # 8 even (int8) groups, 8 odd (bf16) groups
    # w13q block (ge, b): [128 part, kk, w=2, 128] int16 words packing the
    # two m-tiles of the group (lo byte = m0 biased +128, hi = m1 signed)
    w13q = nc.declare_dram_parameter("w13q", [NEG, NB1, P, KQ1, 2, P],
                                     i16, isOutput=False)
    # w13b block (go, b): same layout in bf16 (4KB lines)
    w13b = nc.declare_dram_parameter("w13b", [NEG, NB1, P, KQ1, 2, G1 * P],
                                     bf16, isOutput=False)
    # w2p block (g2, b): [128 part, kk, G2*128] bf16 (4KB lines), s3 folded
    w2p = nc.declare_dram_parameter("w2p", [NG2, NB2, P, KQ2, G2 * P],
                                    bf16, isOutput=False)
    xT = nc.declare_dram_parameter("xT", [P, KD, C], bf16, isOutput=False)
    s1p = nc.declare_dram_parameter("s1p", [P, MF], f32, isOutput=False)
    outT = nc.declare_dram_parameter("outT", [NG2, P, G2, C], bf16,
                                     isOutput=True)

    with tile.TileContext(nc) as tc:
        with (
            tc.tile_pool(name="xpool", bufs=1) as xpool,
            tc.tile_pool(name="hpool", bufs=1) as hpool,
            tc.tile_pool(name="w8pool", bufs=12) as w8pool,
            tc.tile_pool(name="tmpool", bufs=12) as tmpool,
            tc.tile_pool(name="w16pool", bufs=14) as w16pool,
            tc.tile_pool(name="w2pool", bufs=14) as w2pool,
            tc.tile_pool(name="psum", bufs=8, space="PSUM") as psum,
            tc.tile_pool(name="spool", bufs=4) as spool,
            tc.tile_pool(name="opool", bufs=2) as opool,
        ):
            xt = xpool.tile([P, KD, C], bf16)
            nc.sync.dma_start(out=xt[:, 0:4, :], in_=xT[:, 0:4, :])
            nc.scalar.dma_start(out=xt[:, 4:, :], in_=xT[:, 4:, :])
            s1t = xpool.tile([P, MF], f32, name="s1t")
            nc.sync.dma_start(out=s1t[:], in_=s1p[:])
            ht = hpool.tile([P, KF, C], bf16)

            dma_eng = [nc.sync, nc.scalar]
            ndma = 0
            Alu = mybir.AluOpType

            def unpack(dst, srcw):
                # dst [P, 2, KQ1, 2, P] bf16: plane 0 = m0 (lo bytes),
                # plane 1 = m1 (hi bytes).  srcw [P, KQ1, 2, P] int16.
                # All 4 ops on the DVE so the Scalar engine keeps only the
                # silu drain (PSUM banks free without queueing delays).
                tA = tmpool.tile([P, KQ1, 2, P], i16, tag="tA")
                tB = tmpool.tile([P, KQ1, 2, P], i16, tag="tB")
                nc.vector.tensor_scalar(tA[:], srcw[:], 255, None,
                                        Alu.bitwise_and)
                nc.vector.tensor_scalar(tB[:], srcw[:], -256, None,
                                        Alu.bitwise_and)
                nc.vector.tensor_scalar(dst[:, 0], tA[:], 1.0, -128.0,
                                        Alu.mult, Alu.add)
                nc.vector.tensor_scalar(dst[:, 1], tB[:], 1.0 / 256, None,
                                        Alu.mult)

            warm = xpool.tile([P, 256], bf16, name="warm")
            nc.vector.memset(warm[:], 0.0)
            ps_w = psum.tile([P, C], f32, tag="ps", name="ps_warm")
            for i in range(64):
                nc.tensor.matmul(ps_w[:], warm[:, :P], warm[:, :C],
                                 start=True, stop=True)

            # stage 1: hT[f, t] = silu(s1 * (w1q^T xT)) * (w3q^T xT)
            for g in range(NG1):
                ps_g = [psum.tile([P, C], f32, tag="ps", name=f"ps_g{g}_{m}")
                        for m in range(G1)]
                ps_u = [psum.tile([P, C], f32, tag="ps", name=f"ps_u{g}_{m}")
                        for m in range(G1)]
                for b in range(NB1):
                    if g % 2 == 1:
                        w8 = w8pool.tile([P, KQ1, 2, P], i16, tag="w8")
                        dma_eng[ndma % 2].dma_start(out=w8[:],
                                                    in_=w13q[g // 2, b])
                        ndma += 1
                        w16 = w16pool.tile([P, 2, KQ1, 2, P], bf16,
                                           tag="w16")
                        unpack(w16, w8)

                        def lhs1(kk, w_, m, t=w16):
                            return t[:, m, kk, w_, :]
                    else:
                        w16 = w16pool.tile([P, KQ1, 2, G1 * P], bf16,
                                           tag="w16")
                        dma_eng[ndma % 2].dma_start(out=w16[:],
                                                    in_=w13b[g // 2, b])
                        ndma += 1

                        def lhs1(kk, w_, m, t=w16):
                            return t[:, kk, w_, m * P:(m + 1) * P]
                    for kk in range(KQ1):
                        k = b * KQ1 + kk
                        st, sp = (k == 0), (k == KD - 1)
                        for m in range(G1):
                            nc.tensor.matmul(ps_g[m][:], lhs1(kk, 0, m),
                                             xt[:, k, :], start=st, stop=sp)
                            nc.tensor.matmul(ps_u[m][:], lhs1(kk, 1, m),
                                             xt[:, k, :], start=st, stop=sp)
                # drain ops free this group's PSUM banks; schedule them
                # ahead of prefetched unpack work in the engine FIFOs
                with tc.high_priority(offset=200):
                    for m in range(G1):
                        mf = g * G1 + m
                        sig = spool.tile([P, C], f32, tag="sig")
                        nc.scalar.activation(sig[:], ps_g[m][:],
                                             mybir.ActivationFunctionType.Silu,
                                             scale=s1t[:, mf:mf + 1])
                        nc.vector.tensor_tensor(out=ht[:, mf, :], in0=sig[:],
                                                in1=ps_u[m][:],
                                                op=mybir.AluOpType.mult)

            # stage 2: outT[d, t] = w2'^T @ hT
            for g in range(NG2):
                ps_o = [psum.tile([P, C], f32, tag="ps", name=f"ps_o{g}_{m}")
                        for m in range(G2)]
                for b in range(NB2):
                    w2t = w2pool.tile([P, KQ2, G2 * P], bf16, tag="w2")
                    dma_eng[ndma % 2].dma_start(out=w2t[:], in_=w2p[g, b])
                    ndma += 1
                    for kk in range(KQ2):
                        k = b * KQ2 + kk
                        st, sp = (k == 0), (k == KF - 1)
                        for m in range(G2):
                            nc.tensor.matmul(ps_o[m][:],
                                             w2t[:, kk, m * P:(m + 1) * P],
                                             ht[:, k, :], start=st, stop=sp)
                obuf = opool.tile([P, G2, C], bf16, tag="o", name=f"ob{g}")
                with tc.high_priority(offset=200):
                    for m in range(G2):
                        nc.vector.tensor_copy(out=obuf[:, m, :],
                                              in_=ps_o[m][:])
                nc.sync.dma_start(out=outT[g], in_=obuf[:])

    nc.compile()
    return nc


def _route(x2d, gate_w, top_k):
    """Replicates the reference gate on host: returns (sel [T,k], cw [T,k])."""
    logits = x2d @ gate_w                       # [T, E] fp32
    sel = np.argsort(-logits, axis=-1, kind="stable")[:, :top_k]
    vals = np.take_along_axis(logits, sel, axis=-1)
    m = vals.max(axis=-1, keepdims=True)
    ex = np.exp(vals - m)
    cw = ex / ex.sum(axis=-1, keepdims=True)
    return sel, cw


def _quant_pack(w1, w3, w2):
    """Per-expert: int8-quantize the even 256-col groups of w1/w3 (one scale
    per output column), keep odd groups bf16, fold s3 into w2.

    Returns (w13q, w13b, w2p, s1p) in the device DMA layouts."""
    bf16 = _bf16()
    GC = G1 * P  # 256 cols per group
    fcols = np.arange(F)
    even = ((fcols // GC) % 2) == 1
    out = []
    for e in range(E):
        w1e, w3e = w1[e], w3[e]                      # [D, F]
        s1 = np.abs(w1e).max(axis=0) / 127.0         # [F]
        s3 = np.abs(w3e).max(axis=0) / 127.0
        q1 = np.clip(np.rint(w1e / s1), -127, 127).astype(np.int8)
        q3 = np.clip(np.rint(w3e / s3), -127, 127).astype(np.int8)
        # int8 halves on even columns, packed as int16 words pairing the
        # two m-tiles of each group: lo byte = m0 (biased +128), hi = m1
        # (signed).  [w, D, F/2] -> [w, b, kk, p, ge, m, c]
        qs = np.stack([q1[:, even], q3[:, even]], 0)
        qs = qs.reshape(2, NB1, KQ1, P, NG1 // 2, G1, P)
        lo = (qs[..., 0, :].astype(np.int16) + 128).astype(np.uint16) & 0xFF
        hi = qs[..., 1, :].astype(np.int16).astype(np.uint16) << 8
        words = (hi | lo).view(np.int16)           # [w, b, kk, p, ge, c]
        w13q = np.ascontiguousarray(words.transpose(4, 1, 3, 2, 0, 5))
        # bf16 halves on odd columns: h there is true-scale (s1p = s3_eff = 1)
        bs = np.stack([w1e[:, ~even].astype(bf16), w3e[:, ~even].astype(bf16)],
                      0)
        bs = bs.reshape(2, NB1, KQ1, P, NG1 // 2, GC)
        w13b = np.ascontiguousarray(bs.transpose(4, 1, 3, 2, 0, 5))
        # s1 scale vector: even groups s1, odd groups 1.0
        s1v = np.where(even, s1, 1.0).astype(np.float32)
        s1p = np.ascontiguousarray(s1v.reshape(MF, P).T)  # [P, MF]
        # w2' = diag(s3_eff) @ w2, bf16; s3_eff = s3 on even cols, 1 on odd
        s3v = np.where(even, s3, 1.0).astype(np.float32)
        w2f = (w2[e] * s3v[:, None]).astype(bf16)
        w2r = w2f.reshape(NB2, KQ2, P, NG2, G2 * P)
        w2p = np.ascontiguousarray(w2r.transpose(3, 0, 2, 1, 4))
        out.append((w13q, w13b, w2p, s1p))
    return out


def kernel(x, gate_w, w1, w3, w2, top_k):
    from concourse.bass_utils import run_bass_kernel_spmd

    x = np.asarray(x, np.float32)
    gate_w = np.asarray(gate_w, np.float32)
    w1 = np.asarray(w1, np.float32)
    w3 = np.asarray(w3, np.float32)
    w2 = np.asarray(w2, np.float32)
    k = int(top_k)
    bf16 = _bf16()

    x2d = x.reshape(T, D)
    sel, cw = _route(x2d, gate_w, k)

    # token lists per expert
    idx = [np.where((sel == e).any(axis=1))[0] for e in range(E)]
    wgt = []
    for e in range(E):
        m = sel[idx[e]] == e
        wgt.append(cw[idx[e]][m].astype(np.float32))
    counts = np.array([len(i) for i in idx])
    maxc = int(counts.max())
    C = max(140, -(-maxc // 4) * 4)
    n_chunks = 1
    if C > 512:  # capacity overflow: run multiple passes of 512
        C = 512
        n_chunks = -(-maxc // C)

    if C not in _cache:
        _cache[C] = _build(C)
    nc = _cache[C]

    wpacked = _quant_pack(w1, w3, w2)

    out = np.zeros((T, D), np.float32)
    for chunk in range(n_chunks):
        in_maps = []
        for e in range(E):
            ide = idx[e][chunk * C:(chunk + 1) * C]
            xTe = np.zeros((D, C), bf16)
            xTe[:, :len(ide)] = x2d[ide].T.astype(bf16)
            in_maps.append({
                "xT": np.ascontiguousarray(
                    xTe.reshape(KD, P, C).transpose(1, 0, 2)),
                "w13q": wpacked[e][0],
                "w13b": wpacked[e][1],
                "w2p": wpacked[e][2],
                "s1p": wpacked[e][3],
            })
        res = run_bass_kernel_spmd(nc, in_maps, core_ids=list(range(E)))
        global last_results
        last_results = res
        for e in range(E):
            ide = idx[e][chunk * C:(chunk + 1) * C]
            if len(ide) == 0:
                continue
            we = wgt[e][chunk * C:(chunk + 1) * C]
            # outT [NG2, P, G2, C] -> [D, C] with d = g*G2*P + m*P + p
            oTe = res.results[e]["outT"].astype(np.float32)
            oTe = oTe.transpose(0, 2, 1, 3).reshape(D, C)
            # token indices are unique within one expert's list
            out[ide] += we[:, None] * oTe[:, :len(ide)].T

    return out.reshape(B, S, D)
